# revision 64
# baseline (speedup 1.0000x reference)
"""Fused sparse-attention kernel for Trainium2 (8 NeuronCores, data-parallel over batch).

Computation (per batch element b):
    X[s,k]  = enc[b] @ W_enc + dec_proj[b,k] + cov[b,s]*Wcovsum[k] + bias[k]
    T       = tanh(X)
    att[s]  = T @ v_w                      (+ v_b, which cancels in softmax)
    w       = softmax(att masked to s < len[b])
    new_cov = cov + w
Sharding: batch B=32 split 4-per-core across 8 cores; weights replicated.

Key layout/precision choices:
- enc is cast+transposed ON THE HOST to fp8 e4m3 [128p, HC, S] chunk layout, so
  the device does one contiguous full-rate load per batch element (no fp32
  DRAM bounce, no xbar DMA-transpose) and the main GEMM runs fp8 DoubleRow
  (K=256 per pass at 0.5 cyc/row = 2x PE throughput).
- fp8 operands are pre-scaled (enc*0.25, W_enc*16) to dodge e4m3 subnormals;
  the net *4 on psum is undone by the tanh's free scale arg. Host-emulated
  end-to-end relmax vs the fp32 reference: 6.9e-3 (gate 2e-2).
- The additive terms (dec_proj+bias, cov*Wcovsum) stay a bf16 K=2 rank-1
  matmul into the same psum group (R1_FP8 flips them to a K=2 fp8 DoubleRow
  pass at half PE cost, relmax 1.18e-2).
- dec_proj (dec @ W_s, 17 MFLOP total) and Wcovsum are host-computed.

Device pipeline, two s-tiles (=2 psum banks) per step:
  PE:  two accumulation groups into one [128, 2*512] psum pair-tile
  ACT: one tanh over the pair (amortizes the psum-access init cost) -> bf16
  DVE: one paired tensor_tensor T*v multiply (2x bf16 mode), then per s-tile
       tensor_scalar with accum_out for the free-dim reduce (4x mode). The
       obvious single scalar_tensor_tensor runs at 1x (no DVE perf-mode uop),
       so this 3-op split is ~20% faster overall.
Masked-softmax tail per batch in [s_lo=128, s_hi=16] layout: exp on ACT,
iota<len mask fused with the exp multiply on DVE, fp32 sum-matmul + 1/sum
broadcast via two tiny PE matmuls (max-subtraction skipped: |logits| <=
||v||_1 ~ 8, safely inside fp32 exp range; v_b cancels in softmax).
DMA order: batch-0 first quarter + the three first-needed consts go first so
the first matmul issues ~3us in; everything else streams behind it.
"""

import numpy as np
import ml_dtypes

B, S, H, E = 32, 2048, 512, 512
NCORES = 8
BPC = B // NCORES           # batches per core
SLO, SHI = 128, S // 128    # att tile layout: s = 128*j + p  ->  [p, j]
HC = H // 128               # h chunks
BF16 = ml_dtypes.bfloat16

USE_FP8 = True
R1_FP8 = False              # rank-1 terms as fp8 DoubleRow (cheaper PE, more err)
FP8 = ml_dtypes.float8_e4m3fn
ENC_SCALE = 0.25            # enc pre-scale (host)
W_SCALE = 16.0              # W_enc pre-scale (host)
PSUM_SCALE = ENC_SCALE * W_SCALE  # net scale on psum; undone in tanh

_CACHE = {}


def _build_nc():
    import concourse.mybir as mybir
    import concourse.tile as tile
    from concourse import bacc
    from contextlib import ExitStack

    dt = mybir.dt
    F32, BF = dt.float32, dt.bfloat16
    ENC_DT = dt.float8e4 if USE_FP8 else BF

    nc = bacc.Bacc("TRN2", target_bir_lowering=False, debug=False,
                   enable_asserts=False, num_devices=NCORES)

    # ---- DRAM I/O (per-core shapes) ----
    # encT[b, p, (j, c, si)] = enc[b, 128j+si, 128c+p]  (pre-scaled when fp8):
    # j-granular slices stay >=512B-contiguous per partition => full DMA rate
    encT = nc.dram_tensor("encT", [BPC, 128, SHI * HC * 128], ENC_DT,
                          kind="ExternalInput").ap()
    # wblob: wenc chunk c at cols [c*H, (c+1)*H): wenc[c][p, k] = W[128c+p, k]
    wblob = nc.dram_tensor("wblob", [128, HC * H], ENC_DT,
                           kind="ExternalInput").ap()
    # f32 blob: [iota (SHI) | lens (BPC)]
    fblob = nc.dram_tensor("fblob", [SLO, SHI + BPC], F32,
                           kind="ExternalInput").ap()
    if R1_FP8:
        r1 = nc.dram_tensor("r1", [1, 2 * BPC * (S + H)], ENC_DT,
                            kind="ExternalInput").ap()
    else:
        # [lhs (ones,cov) BPC*S | rhs ((dec_proj+b)*PS, Wcovsum*PS) BPC*H]
        r1 = nc.dram_tensor("r1", [2, BPC * (S + H)], BF,
                            kind="ExternalInput").ap()
    vbc = nc.dram_tensor("vbc", [128, 2 * H], BF, kind="ExternalInput").ap()
    # unnormalized masked exp(att); the softmax normalize (sum + exact
    # divide) and the cov add are a host-side elementwise epilogue
    att_out = nc.dram_tensor("att_out", [BPC, SLO, SHI], F32, kind="ExternalOutput").ap()

    AF = mybir.ActivationFunctionType
    OP = mybir.AluOpType
    DR = mybir.MatmulPerfMode.DoubleRow

    with tile.TileContext(nc) as tc, ExitStack() as ctx:
        consts = ctx.enter_context(tc.tile_pool(name="consts", bufs=1))
        encp = ctx.enter_context(tc.tile_pool(name="encp", bufs=2))
        tpool = ctx.enter_context(tc.tile_pool(name="tpool", bufs=4))
        spool = ctx.enter_context(tc.tile_pool(name="spool", bufs=3))
        small = ctx.enter_context(tc.tile_pool(name="small", bufs=2))
        attp = ctx.enter_context(tc.tile_pool(name="attp", bufs=4))
        ppm = ctx.enter_context(tc.tile_pool(name="ppm", bufs=3, space="PSUM"))

        # ---- DMA order: batch-0 first quarter, then the first-needed consts,
        # then the rest of batch 0, then the remaining consts. Input DMAs ride
        # the SP (sync) queue; DMA_ENGINES serialize roughly in request order
        # so this gets the first matmul issued ~3us in. ----
        def enc_tile():
            return encp.tile([128, SHI, HC * 128], ENC_DT, tag="enc",
                             name="enc_t")

        def enc_load(e_t, b, lo, hi):
            src = encT[b].rearrange("p (j x) -> p j x", j=SHI)
            nc.sync.dma_start(e_t[:, lo:hi, :], src[:, lo:hi, :])

        # first-needed consts ride the SP HWDGE queue (the ACT queue is
        # blocked by its 1.3us activation-table load at program start, and
        # the Pool SWDGE path has high fixed latency), smallest first, so the
        # first matmul can go ~2.5us in.
        if R1_FP8:
            r1_sb = consts.tile([1, 2 * BPC * (S + H)], ENC_DT, tag="r1")
        else:
            r1_sb = consts.tile([2, BPC * (S + H)], BF, tag="r1")
        nc.sync.dma_start(r1_sb[:], r1[:])
        wb_sb = consts.tile([128, HC * H], ENC_DT, tag="wblob")
        nc.sync.dma_start(wb_sb[:], wblob[:])
        e0 = enc_tile()
        enc_load(e0, 0, 0, 2)
        if R1_FP8:
            r1l3 = r1_sb[:, 0:2 * BPC * S].rearrange("p (x c) -> p x c", x=2)
            r1r3 = r1_sb[:, 2 * BPC * S:].rearrange("p (x c) -> p x c", x=2)
        else:
            r1lhs_sb = r1_sb[:, 0:BPC * S]
            r1rhs_sb = r1_sb[:, BPC * S:]

        enc_load(e0, 0, 2, 6)
        enc_load(e0, 0, 6, 16)

        vbc_sb = consts.tile([128, 2 * H], BF, tag="vbc")
        nc.gpsimd.dma_start(vbc_sb[:], vbc[:])
        fb_sb = consts.tile([SLO, SHI + BPC], F32, tag="fblob")
        nc.gpsimd.dma_start(fb_sb[:], fblob[:])

        iota_sb = fb_sb[:, 0:SHI]
        lens_sb = fb_sb[:, SHI:SHI + BPC]



        def load_batch(b):
            e_t = enc_tile()
            enc_load(e_t, b, 0, 8)
            enc_load(e_t, b, 8, 16)
            return e_t

        pre = {0: e0}
        wb3 = wb_sb[:].rearrange("p (c k) -> p c k", c=HC)

        # ---- main loop: two s-tiles (2 psum banks) per step ----
        for b in range(BPC):
            enc_t = pre.pop(b)
            if b + 1 < BPC:
                pre[b + 1] = load_batch(b + 1)

            att_t = attp.tile([SLO, SHI], F32, tag="att")
            expt = small.tile([SLO, SHI], F32, tag="expt")
            mexp = small.tile([SLO, SHI], F32, tag="mexp")

            # masked-exp tail for columns [lo, hi): emitted in halves, the
            # first half mid-stream so ACT executes it between tanhs (range-
            # tracked deps let it fire once ts j<hi are done) instead of
            # serializing the whole chain after the last tanh. The softmax
            # sum+divide and the cov add are a host-side epilogue on mexp.
            def tail(lo, hi):
                sl = slice(lo, hi)
                nc.scalar.activation(expt[:, sl], att_t[:, sl], AF.Exp)
                nc.vector.scalar_tensor_tensor(
                    out=mexp[:, sl], in0=iota_sb[:, sl],
                    scalar=lens_sb[:, b:b + 1],
                    in1=expt[:, sl], op0=OP.is_lt, op1=OP.mult,
                )
                nc.sync.dma_start(att_out[b][:, sl], mexp[:, sl])

            enc4 = enc_t[:].rearrange("p j (c y) -> p j c y", c=HC)
            NQ = 2  # s-tiles per psum tile (2 banks; 4 bufs = all 8 banks)
            for j0 in range(0, SHI, NQ):
                if j0 == SHI - NQ:
                    tail(0, 8)
                # the very first pair runs its ACT/DVE stages per single
                # s-tile: shorter pipeline-fill at the head
                grain = 1 if (b == 0 and j0 == 0) or \
                             (b == BPC - 1 and j0 == SHI - NQ) else NQ
                ps = ppm.tile([128, NQ * H], F32, tag="x")
                # rank-1s of all groups first: they depend only on the tiny
                # r1 blob, so at the head PE starts (and ramps) before enc lands
                for jj in range(NQ):
                    j = j0 + jj
                    psl = ps[:, jj * H:(jj + 1) * H]
                    if R1_FP8:
                        nc.tensor.matmul(
                            psl,
                            r1l3[:, :, b * S + j * 128: b * S + (j + 1) * 128],
                            r1r3[:, :, b * H:(b + 1) * H],
                            start=True, stop=False, perf_mode=DR,
                        )
                    else:
                        nc.tensor.matmul(
                            psl,
                            r1lhs_sb[:, b * S + j * 128: b * S + (j + 1) * 128],
                            r1rhs_sb[:, b * H:(b + 1) * H],
                            start=True, stop=False,
                        )
                for jj in range(NQ):
                    j = j0 + jj
                    psl = ps[:, jj * H:(jj + 1) * H]
                    if USE_FP8:
                        for c in range(0, HC, 2):
                            nc.tensor.matmul(
                                psl,
                                enc4[:, j, c:c + 2, :],
                                wb3[:, c:c + 2, :],
                                start=False, stop=(c + 2 == HC),
                                perf_mode=DR,
                            )
                    else:
                        for c in range(HC):
                            nc.tensor.matmul(
                                psl,
                                enc4[:, j, c, :],
                                wb3[:, c, :],
                                start=False, stop=(c == HC - 1),
                            )
                t_t = tpool.tile([128, NQ * H], BF, tag="t")
                tanh_scale = 1.0 / PSUM_SCALE if USE_FP8 else 1.0
                scr = spool.tile([128, NQ * H], BF, tag="scr")
                for g0 in range(0, NQ, grain):
                    sl = slice(g0 * H, (g0 + grain) * H)
                    nc.scalar.activation(t_t[:, sl], ps[:, sl], AF.Tanh,
                                         scale=tanh_scale)
                    nc.vector.tensor_tensor(scr[:, sl], t_t[:, sl],
                                            vbc_sb[:, 0:grain * H], OP.mult)
                    for jj in range(g0, g0 + grain):
                        j = j0 + jj
                        scr2 = spool.tile([128, H], BF, tag="scr2")
                        nc.vector.tensor_scalar(
                            scr2[:], scr[:, jj * H:(jj + 1) * H], 1.0, None,
                            OP.mult, OP.add, accum_out=att_t[:, j:j + 1],
                        )

            tail(8, SHI)

    nc.compile()
    return nc


def _get_nc():
    if "nc" not in _CACHE:
        _CACHE["nc"] = _build_nc()
    return _CACHE["nc"]


def _prep_in_maps(dec_input, enc_output, text_lengths, coverage_vector, W, b, v_w):
    enc = np.asarray(enc_output, dtype=np.float32)
    dec = np.asarray(dec_input, dtype=np.float32).reshape(B, E)
    cov = np.asarray(coverage_vector, dtype=np.float32)
    W = np.asarray(W, dtype=np.float32)
    b = np.asarray(b, dtype=np.float32)
    v_w = np.asarray(v_w, dtype=np.float32)
    lens_f = np.asarray(text_lengths).astype(np.float32)

    enc_dt = FP8 if USE_FP8 else BF16
    ps = PSUM_SCALE if USE_FP8 else 1.0
    es = ENC_SCALE if USE_FP8 else 1.0
    ws = W_SCALE if USE_FP8 else 1.0

    # enc^T layout [B, 128p, SHI, HC, 128s], host-cast (+pre-scale for fp8)
    encT = (enc * es if USE_FP8 else enc).reshape(B, SHI, 128, HC, 128) \
        .transpose(0, 4, 1, 3, 2)
    encT = np.ascontiguousarray(encT).astype(enc_dt) \
        .reshape(B, 128, SHI * HC * 128)

    wenc = W[:H] * ws                                  # (H, H)
    wblob = np.ascontiguousarray(
        wenc.reshape(HC, 128, H).transpose(1, 0, 2).reshape(128, HC * H)
    ).astype(enc_dt)

    dec_proj = dec @ W[H:H + E] + b                    # (B, H)
    wcovsum = W[H + E:].sum(axis=0, dtype=np.float32)  # (H,)

    vbc = np.ascontiguousarray(np.broadcast_to(
        np.concatenate([v_w] * 2).astype(BF16), (128, 2 * H)))
    iota = (np.arange(SLO, dtype=np.float32)[:, None]
            + 128.0 * np.arange(SHI, dtype=np.float32)[None, :])

    in_maps = []
    for core in range(NCORES):
        sl = slice(core * BPC, (core + 1) * BPC)

        fblob = np.empty((SLO, SHI + BPC), np.float32)
        fblob[:, 0:SHI] = iota
        fblob[:, SHI:SHI + BPC] = lens_f[sl][None, :]

        if R1_FP8:
            r1 = np.empty((1, 2, BPC * (S + H)), np.float32)
            r1[0, 0, :BPC * S] = 1.0
            r1[0, 1, :BPC * S] = cov[sl].reshape(-1)
            r1[0, 0, BPC * S:] = (dec_proj[sl] * ps).reshape(-1)
            r1[0, 1, BPC * S:] = np.broadcast_to(wcovsum * ps, (BPC, H)).reshape(-1)
            # interleave: [lhs-pair | rhs-pair] as separate x-major blocks
            r1b = np.empty((1, 2 * BPC * (S + H)), np.float32)
            r1b[0, :2 * BPC * S] = r1[0, :, :BPC * S].reshape(-1)
            r1b[0, 2 * BPC * S:] = r1[0, :, BPC * S:].reshape(-1)
            r1 = r1b.astype(enc_dt)
        else:
            r1 = np.empty((2, BPC * (S + H)), np.float32)
            r1[0, :BPC * S] = 1.0
            r1[1, :BPC * S] = cov[sl].reshape(-1)
            r1[0, BPC * S:] = (dec_proj[sl] * ps).reshape(-1)
            r1[1, BPC * S:] = np.broadcast_to(wcovsum * ps, (BPC, H)).reshape(-1)
            r1 = r1.astype(BF16)

        in_maps.append({
            "encT": encT[sl],
            "wblob": wblob,
            "fblob": fblob,
            "r1": r1,
            "vbc": vbc,
        })
    return in_maps


def kernel(dec_input, enc_output, text_lengths, coverage_vector, W, b, v_w, v_b):
    from concourse.bass_utils import run_bass_kernel_spmd

    nc = _get_nc()
    in_maps = _prep_in_maps(dec_input, enc_output, text_lengths,
                            coverage_vector, W, b, v_w)
    res = run_bass_kernel_spmd(nc, in_maps, core_ids=list(range(NCORES)))

    att = np.empty((B, S), np.float32)
    for core in range(NCORES):
        r = res.results[core]
        att[core * BPC:(core + 1) * BPC] = \
            r["att_out"].transpose(0, 2, 1).reshape(BPC, S)
    att /= att.sum(axis=1, keepdims=True, dtype=np.float32)
    ncov = np.asarray(coverage_vector, dtype=np.float32) + att
    return att, ncov


# revision 66
# speedup vs baseline: 1.0156x; 1.0156x over previous
"""Fused sparse-attention kernel for Trainium2 (8 NeuronCores, data-parallel over batch).

Computation (per batch element b):
    X[s,k]  = enc[b] @ W_enc + dec_proj[b,k] + cov[b,s]*Wcovsum[k] + bias[k]
    T       = tanh(X)
    att[s]  = T @ v_w                      (+ v_b, which cancels in softmax)
    w       = softmax(att masked to s < len[b])
    new_cov = cov + w
Sharding: batch B=32 split 4-per-core across 8 cores; weights replicated.

Key layout/precision choices:
- enc is cast+transposed ON THE HOST to fp8 e4m3 [128p, HC, S] chunk layout, so
  the device does one contiguous full-rate load per batch element (no fp32
  DRAM bounce, no xbar DMA-transpose) and the main GEMM runs fp8 DoubleRow
  (K=256 per pass at 0.5 cyc/row = 2x PE throughput).
- fp8 operands are pre-scaled (enc*0.25, W_enc*16) to dodge e4m3 subnormals;
  the net *4 on psum is undone by the tanh's free scale arg. Host-emulated
  end-to-end relmax vs the fp32 reference: 6.9e-3 (gate 2e-2).
- The additive terms (dec_proj+bias, cov*Wcovsum) stay a bf16 K=2 rank-1
  matmul into the same psum group (R1_FP8 flips them to a K=2 fp8 DoubleRow
  pass at half PE cost, relmax 1.18e-2).
- dec_proj (dec @ W_s, 17 MFLOP total) and Wcovsum are host-computed.

Device pipeline, two s-tiles (=2 psum banks) per step:
  PE:  two accumulation groups into one [128, 2*512] psum pair-tile
  ACT: one tanh over the pair (amortizes the psum-access init cost) -> bf16
  DVE: one paired tensor_tensor T*v multiply (2x bf16 mode), then per s-tile
       tensor_scalar with accum_out for the free-dim reduce (4x mode). The
       obvious single scalar_tensor_tensor runs at 1x (no DVE perf-mode uop),
       so this 3-op split is ~20% faster overall.
Masked-softmax tail per batch in [s_lo=128, s_hi=16] layout: exp on ACT,
iota<len mask fused with the exp multiply on DVE, fp32 sum-matmul + 1/sum
broadcast via two tiny PE matmuls (max-subtraction skipped: |logits| <=
||v||_1 ~ 8, safely inside fp32 exp range; v_b cancels in softmax).
DMA order: batch-0 first quarter + the three first-needed consts go first so
the first matmul issues ~3us in; everything else streams behind it.
"""

import numpy as np
import ml_dtypes

B, S, H, E = 32, 2048, 512, 512
NCORES = 8
BPC = B // NCORES           # batches per core
SLO, SHI = 128, S // 128    # att tile layout: s = 128*j + p  ->  [p, j]
HC = H // 128               # h chunks
BF16 = ml_dtypes.bfloat16

USE_FP8 = True
R1_FP8 = False              # rank-1 terms as fp8 DoubleRow (cheaper PE, more err)
FP8 = ml_dtypes.float8_e4m3fn
ENC_SCALE = 0.25            # enc pre-scale (host)
W_SCALE = 16.0              # W_enc pre-scale (host)
PSUM_SCALE = ENC_SCALE * W_SCALE  # net scale on psum; undone in tanh

_CACHE = {}


def _build_nc():
    import concourse.mybir as mybir
    import concourse.tile as tile
    from concourse import bacc
    from contextlib import ExitStack

    dt = mybir.dt
    F32, BF = dt.float32, dt.bfloat16
    ENC_DT = dt.float8e4 if USE_FP8 else BF

    nc = bacc.Bacc("TRN2", target_bir_lowering=False, debug=False,
                   enable_asserts=False, num_devices=NCORES)

    # ---- DRAM I/O (per-core shapes) ----
    # encT[b, p, (j, c, si)] = enc[b, 128j+si, 128c+p]  (pre-scaled when fp8):
    # j-granular slices stay >=512B-contiguous per partition => full DMA rate
    encT = nc.dram_tensor("encT", [BPC, 128, SHI * HC * 128], ENC_DT,
                          kind="ExternalInput").ap()
    # wblob: wenc chunk c at cols [c*H, (c+1)*H): wenc[c][p, k] = W[128c+p, k]
    wblob = nc.dram_tensor("wblob", [128, HC * H], ENC_DT,
                           kind="ExternalInput").ap()
    # f32 blob: [iota (SHI) | lens (BPC)]
    fblob = nc.dram_tensor("fblob", [SLO, SHI + BPC], F32,
                           kind="ExternalInput").ap()
    if R1_FP8:
        r1 = nc.dram_tensor("r1", [1, 2 * BPC * (S + H)], ENC_DT,
                            kind="ExternalInput").ap()
    else:
        # [lhs (ones,cov) BPC*S | rhs ((dec_proj+b)*PS, Wcovsum*PS) BPC*H]
        r1 = nc.dram_tensor("r1", [2, BPC * (S + H)], BF,
                            kind="ExternalInput").ap()
    vbc = nc.dram_tensor("vbc", [128, 2 * H], BF, kind="ExternalInput").ap()
    # unnormalized masked exp(att); the softmax normalize (sum + exact
    # divide) and the cov add are a host-side elementwise epilogue
    att_out = nc.dram_tensor("att_out", [BPC, SLO, SHI], F32, kind="ExternalOutput").ap()

    AF = mybir.ActivationFunctionType
    OP = mybir.AluOpType
    DR = mybir.MatmulPerfMode.DoubleRow

    with tile.TileContext(nc) as tc, ExitStack() as ctx:
        consts = ctx.enter_context(tc.tile_pool(name="consts", bufs=1))
        encp = ctx.enter_context(tc.tile_pool(name="encp", bufs=2))
        tpool = ctx.enter_context(tc.tile_pool(name="tpool", bufs=4))
        spool = ctx.enter_context(tc.tile_pool(name="spool", bufs=3))
        small = ctx.enter_context(tc.tile_pool(name="small", bufs=2))
        attp = ctx.enter_context(tc.tile_pool(name="attp", bufs=4))
        ppm = ctx.enter_context(tc.tile_pool(name="ppm", bufs=3, space="PSUM"))

        # ---- DMA order: batch-0 first quarter, then the first-needed consts,
        # then the rest of batch 0, then the remaining consts. Input DMAs ride
        # the SP (sync) queue; DMA_ENGINES serialize roughly in request order
        # so this gets the first matmul issued ~3us in. ----
        def enc_tile():
            return encp.tile([128, SHI, HC * 128], ENC_DT, tag="enc",
                             name="enc_t")

        def enc_load(e_t, b, lo, hi):
            src = encT[b].rearrange("p (j x) -> p j x", j=SHI)
            nc.sync.dma_start(e_t[:, lo:hi, :], src[:, lo:hi, :])

        # first-needed consts ride the SP HWDGE queue (the ACT queue is
        # blocked by its 1.3us activation-table load at program start, and
        # the Pool SWDGE path has high fixed latency), smallest first, so the
        # first matmul can go ~2.5us in.
        if R1_FP8:
            r1_sb = consts.tile([1, 2 * BPC * (S + H)], ENC_DT, tag="r1")
        else:
            r1_sb = consts.tile([2, BPC * (S + H)], BF, tag="r1")
        nc.sync.dma_start(r1_sb[:], r1[:])
        wb_sb = consts.tile([128, HC * H], ENC_DT, tag="wblob")
        nc.sync.dma_start(wb_sb[:], wblob[:])
        e0 = enc_tile()
        enc_load(e0, 0, 0, 2)
        if R1_FP8:
            r1l3 = r1_sb[:, 0:2 * BPC * S].rearrange("p (x c) -> p x c", x=2)
            r1r3 = r1_sb[:, 2 * BPC * S:].rearrange("p (x c) -> p x c", x=2)
        else:
            r1lhs_sb = r1_sb[:, 0:BPC * S]
            r1rhs_sb = r1_sb[:, BPC * S:]

        enc_load(e0, 0, 2, 6)
        enc_load(e0, 0, 6, 16)

        vbc_sb = consts.tile([128, 2 * H], BF, tag="vbc")
        nc.gpsimd.dma_start(vbc_sb[:], vbc[:])
        fb_sb = consts.tile([SLO, SHI + BPC], F32, tag="fblob")
        nc.gpsimd.dma_start(fb_sb[:], fblob[:])

        iota_sb = fb_sb[:, 0:SHI]
        lens_sb = fb_sb[:, SHI:SHI + BPC]



        def load_batch(b):
            e_t = enc_tile()
            enc_load(e_t, b, 0, 8)
            enc_load(e_t, b, 8, 16)
            return e_t

        pre = {0: e0}
        wb3 = wb_sb[:].rearrange("p (c k) -> p c k", c=HC)

        # ---- main loop: two s-tiles (2 psum banks) per step ----
        for b in range(BPC):
            enc_t = pre.pop(b)
            if b + 1 < BPC:
                pre[b + 1] = load_batch(b + 1)

            att_t = attp.tile([SLO, SHI], F32, tag="att")
            expt = small.tile([SLO, SHI], F32, tag="expt")
            mexp = small.tile([SLO, SHI], F32, tag="mexp")

            # masked-exp tail for columns [lo, hi): emitted in halves, the
            # first half mid-stream so ACT executes it between tanhs (range-
            # tracked deps let it fire once ts j<hi are done) instead of
            # serializing the whole chain after the last tanh. The softmax
            # sum+divide and the cov add are a host-side epilogue on mexp.
            def tail(lo, hi):
                sl = slice(lo, hi)
                nc.scalar.activation(expt[:, sl], att_t[:, sl], AF.Exp)
                nc.vector.scalar_tensor_tensor(
                    out=mexp[:, sl], in0=iota_sb[:, sl],
                    scalar=lens_sb[:, b:b + 1],
                    in1=expt[:, sl], op0=OP.is_lt, op1=OP.mult,
                )
                nc.sync.dma_start(att_out[b][:, sl], mexp[:, sl])

            enc4 = enc_t[:].rearrange("p j (c y) -> p j c y", c=HC)
            NQ = 2  # s-tiles per psum tile (2 banks; 4 bufs = all 8 banks)
            for j0 in range(0, SHI, NQ):
                # the very first pair runs its ACT/DVE stages per single
                # s-tile: shorter pipeline-fill at the head
                grain = 1 if (b == 0 and j0 == 0) or \
                             (b == BPC - 1 and j0 == SHI - NQ) else NQ
                ps = ppm.tile([128, NQ * H], F32, tag="x")
                # rank-1s of all groups first: they depend only on the tiny
                # r1 blob, so at the head PE starts (and ramps) before enc lands
                for jj in range(NQ):
                    j = j0 + jj
                    psl = ps[:, jj * H:(jj + 1) * H]
                    if R1_FP8:
                        nc.tensor.matmul(
                            psl,
                            r1l3[:, :, b * S + j * 128: b * S + (j + 1) * 128],
                            r1r3[:, :, b * H:(b + 1) * H],
                            start=True, stop=False, perf_mode=DR,
                        )
                    else:
                        nc.tensor.matmul(
                            psl,
                            r1lhs_sb[:, b * S + j * 128: b * S + (j + 1) * 128],
                            r1rhs_sb[:, b * H:(b + 1) * H],
                            start=True, stop=False,
                        )
                for jj in range(NQ):
                    j = j0 + jj
                    psl = ps[:, jj * H:(jj + 1) * H]
                    if USE_FP8:
                        for c in range(0, HC, 2):
                            nc.tensor.matmul(
                                psl,
                                enc4[:, j, c:c + 2, :],
                                wb3[:, c:c + 2, :],
                                start=False, stop=(c + 2 == HC),
                                perf_mode=DR,
                            )
                    else:
                        for c in range(HC):
                            nc.tensor.matmul(
                                psl,
                                enc4[:, j, c, :],
                                wb3[:, c, :],
                                start=False, stop=(c == HC - 1),
                            )
                t_t = tpool.tile([128, NQ * H], BF, tag="t")
                tanh_scale = 1.0 / PSUM_SCALE if USE_FP8 else 1.0
                scr = spool.tile([128, NQ * H], BF, tag="scr")
                for g0 in range(0, NQ, grain):
                    sl = slice(g0 * H, (g0 + grain) * H)
                    nc.scalar.activation(t_t[:, sl], ps[:, sl], AF.Tanh,
                                         scale=tanh_scale)
                    nc.vector.tensor_tensor(scr[:, sl], t_t[:, sl],
                                            vbc_sb[:, 0:grain * H], OP.mult)
                    for jj in range(g0, g0 + grain):
                        j = j0 + jj
                        scr2 = spool.tile([128, H], BF, tag="scr2")
                        nc.vector.tensor_scalar(
                            scr2[:], scr[:, jj * H:(jj + 1) * H], 1.0, None,
                            OP.mult, OP.add, accum_out=att_t[:, j:j + 1],
                        )

            if b == BPC - 1:
                tail(0, 8)
                tail(8, SHI)
            else:
                tail(0, SHI)

    nc.compile()
    return nc


def _get_nc():
    if "nc" not in _CACHE:
        _CACHE["nc"] = _build_nc()
    return _CACHE["nc"]


def _prep_in_maps(dec_input, enc_output, text_lengths, coverage_vector, W, b, v_w):
    enc = np.asarray(enc_output, dtype=np.float32)
    dec = np.asarray(dec_input, dtype=np.float32).reshape(B, E)
    cov = np.asarray(coverage_vector, dtype=np.float32)
    W = np.asarray(W, dtype=np.float32)
    b = np.asarray(b, dtype=np.float32)
    v_w = np.asarray(v_w, dtype=np.float32)
    lens_f = np.asarray(text_lengths).astype(np.float32)

    enc_dt = FP8 if USE_FP8 else BF16
    ps = PSUM_SCALE if USE_FP8 else 1.0
    es = ENC_SCALE if USE_FP8 else 1.0
    ws = W_SCALE if USE_FP8 else 1.0

    # enc^T layout [B, 128p, SHI, HC, 128s], host-cast (+pre-scale for fp8)
    encT = (enc * es if USE_FP8 else enc).reshape(B, SHI, 128, HC, 128) \
        .transpose(0, 4, 1, 3, 2)
    encT = np.ascontiguousarray(encT).astype(enc_dt) \
        .reshape(B, 128, SHI * HC * 128)

    wenc = W[:H] * ws                                  # (H, H)
    wblob = np.ascontiguousarray(
        wenc.reshape(HC, 128, H).transpose(1, 0, 2).reshape(128, HC * H)
    ).astype(enc_dt)

    dec_proj = dec @ W[H:H + E] + b                    # (B, H)
    wcovsum = W[H + E:].sum(axis=0, dtype=np.float32)  # (H,)

    vbc = np.ascontiguousarray(np.broadcast_to(
        np.concatenate([v_w] * 2).astype(BF16), (128, 2 * H)))
    iota = (np.arange(SLO, dtype=np.float32)[:, None]
            + 128.0 * np.arange(SHI, dtype=np.float32)[None, :])

    in_maps = []
    for core in range(NCORES):
        sl = slice(core * BPC, (core + 1) * BPC)

        fblob = np.empty((SLO, SHI + BPC), np.float32)
        fblob[:, 0:SHI] = iota
        fblob[:, SHI:SHI + BPC] = lens_f[sl][None, :]

        if R1_FP8:
            r1 = np.empty((1, 2, BPC * (S + H)), np.float32)
            r1[0, 0, :BPC * S] = 1.0
            r1[0, 1, :BPC * S] = cov[sl].reshape(-1)
            r1[0, 0, BPC * S:] = (dec_proj[sl] * ps).reshape(-1)
            r1[0, 1, BPC * S:] = np.broadcast_to(wcovsum * ps, (BPC, H)).reshape(-1)
            # interleave: [lhs-pair | rhs-pair] as separate x-major blocks
            r1b = np.empty((1, 2 * BPC * (S + H)), np.float32)
            r1b[0, :2 * BPC * S] = r1[0, :, :BPC * S].reshape(-1)
            r1b[0, 2 * BPC * S:] = r1[0, :, BPC * S:].reshape(-1)
            r1 = r1b.astype(enc_dt)
        else:
            r1 = np.empty((2, BPC * (S + H)), np.float32)
            r1[0, :BPC * S] = 1.0
            r1[1, :BPC * S] = cov[sl].reshape(-1)
            r1[0, BPC * S:] = (dec_proj[sl] * ps).reshape(-1)
            r1[1, BPC * S:] = np.broadcast_to(wcovsum * ps, (BPC, H)).reshape(-1)
            r1 = r1.astype(BF16)

        in_maps.append({
            "encT": encT[sl],
            "wblob": wblob,
            "fblob": fblob,
            "r1": r1,
            "vbc": vbc,
        })
    return in_maps


def kernel(dec_input, enc_output, text_lengths, coverage_vector, W, b, v_w, v_b):
    from concourse.bass_utils import run_bass_kernel_spmd

    nc = _get_nc()
    in_maps = _prep_in_maps(dec_input, enc_output, text_lengths,
                            coverage_vector, W, b, v_w)
    res = run_bass_kernel_spmd(nc, in_maps, core_ids=list(range(NCORES)))

    att = np.empty((B, S), np.float32)
    for core in range(NCORES):
        r = res.results[core]
        att[core * BPC:(core + 1) * BPC] = \
            r["att_out"].transpose(0, 2, 1).reshape(BPC, S)
    att /= att.sum(axis=1, keepdims=True, dtype=np.float32)
    ncov = np.asarray(coverage_vector, dtype=np.float32) + att
    return att, ncov
